# revision 22
# baseline (speedup 1.0000x reference)
"""DeepLabCE loss (log-softmax + smooth-label weighted sum + top-70% mean)
on 8 Trainium2 NeuronCores.

Sharding: core i <- (b = i//2, h-half = i%2) slice of [B=4, C=19, H=512, W=1024]
inputs, i.e. each core streams a [19, 262144]-pixel shard of logits and
smooth_labels (~40 MB/core).  Per-pixel losses are computed on-device
(memory-bound streaming); the final top-k selection over the 8 MB loss
vector is done on the host during unsharding.

Math per pixel p:  loss[p] = s1[p]*lse[p] - s2[p]
  lse = log(sum_c exp(logit_c))          (logits ~ N(0,1): no max-sub needed)
  s1  = sum_c smooth_c * w_c
  s2  = sum_c smooth_c * w_c * logit_c
Per-class C-reduction runs on the PE via bf16 identity-matmul accumulation
into fp32 PSUM; exp on ACT; (smooth*w) and (smooth*w*logit) on DVE.
"""

import numpy as np

B, C, H, W = 4, 19, 512, 1024
NCORES = 8
NPIX = B * H * W                      # 2097152
PIX_PER_CORE = NPIX // NCORES        # 262144
P = 128                              # SBUF partitions
F = 512                              # free-dim per tile (one fp32 PSUM bank)
NT = PIX_PER_CORE // (P * F)         # 4 tile positions per core
K_TOP = int(0.7 * NPIX)              # same formula as the reference

_cache = {}


def build_nc(repeat=1):
    import concourse.bacc as bacc
    import concourse.mybir as mybir
    from concourse import tile

    dt = mybir.dt
    AF = mybir.ActivationFunctionType
    OP = mybir.AluOpType

    # Bacc (not raw Bass): its finalize() pipeline runs
    # generate_event_semaphores, which splits multi-sem waits to satisfy the
    # TRN2 1-wait-per-instruction constraint walrus enforces.
    class _Bacc(bacc.Bacc):
        def insert_act_table_loads(self):
            # Steer Exp and Ln to the one table set holding BOTH so the
            # kernel needs a single ACT_TABLE_LOAD instead of reloading on
            # every exp-batch/log alternation.  act_func_set_id is the
            # positional index into act_info.json's act_func_sets, so the
            # list order must be preserved — mask Exp/Ln out of every other
            # set instead of reordering.
            import bass_rust as _br
            from concourse.hw_specs import get_activation_tables

            AF = mybir.ActivationFunctionType
            both = {AF.Exp, AF.Ln}
            tables = []
            for name, fns in get_activation_tables(self.m.arch).items():
                if name != "natural_log_exp_and_others":
                    fns = fns - both
                tables.append((name, fns))
            _br.insert_act_table_loads(self, tables)

    nc = _Bacc(None)
    lg = nc.dram_tensor("lg", [C, PIX_PER_CORE], dt.float32, kind="ExternalInput")
    sm = nc.dram_tensor("sm", [C, PIX_PER_CORE], dt.float32, kind="ExternalInput")
    wrep = nc.dram_tensor("wrep", [P, C], dt.float32, kind="ExternalInput")
    ident = nc.dram_tensor("ident", [P, P], dt.bfloat16, kind="ExternalInput")
    # bf16 loss output: halves output-DMA bytes; the top-70% mean over 1.47M
    # values absorbs the rounding (adds ~1e-6 relative error)
    loss = nc.dram_tensor("loss", [PIX_PER_CORE], dt.bfloat16, kind="ExternalOutput")

    # Tile positions: the last ones shrink so the end-of-kernel dependency
    # chain (last input DMA -> exp/mul/matmul -> log -> loss -> output DMA)
    # runs on a small tile instead of a full 64K-pixel one.
    FS = [512, 512, 512, 384, 128]
    assert sum(FS) * P == PIX_PER_CORE

    # c-groups: one input DMA per (tensor, position, group) instead of per
    # (position, c) — the SP sequencer's per-dma_start issue time otherwise
    # rivals the DMA engines themselves.
    CG = 4
    groups = [list(range(g, min(g + CG, C))) for g in range(0, C, CG)]
    # smaller lead group for the very first position: the first input DMA's
    # SP issue time scales with descriptor count, so a slim lead group starts
    # the transfer stream sooner
    groups_first = [[0], [1, 2, 3]] + groups[1:]

    with tile.TileContext(nc) as tc:
        with (
            tc.tile_pool(name="const", bufs=1) as constp,
            tc.tile_pool(name="lp", bufs=5) as lp,
            tc.tile_pool(name="sp", bufs=5) as sp,
            tc.tile_pool(name="ep", bufs=6) as ep,
            tc.tile_pool(name="swp", bufs=6) as swp,
            tc.tile_pool(name="mp", bufs=6) as mp,
            tc.tile_pool(name="outp", bufs=3) as outp,
            tc.tile_pool(name="psum", bufs=2, space="PSUM") as psump,
        ):
            wr_t = constp.tile([P, C], dt.float32, tag="wrep")
            nc.gpsimd.dma_start(wr_t[:], wrep[:])
            id_t = constp.tile([P, P], dt.bfloat16, tag="ident")
            nc.gpsimd.dma_start(id_t[:], ident[:])

          for _rep in range(repeat):
            pix_off = 0
            for t, Fp in enumerate(FS):
                npx = P * Fp
                # [P, C, Fp] view of this position's pixels for each tensor
                lgv = lg[:, pix_off : pix_off + npx].rearrange(
                    "c (p f) -> p c f", p=P
                )
                smv = sm[:, pix_off : pix_off + npx].rearrange(
                    "c (p f) -> p c f", p=P
                )
                lov = loss[pix_off : pix_off + npx].rearrange("(p f) -> p f", p=P)

                acc_e = psump.tile([P, F], dt.float32, tag="acc_e")
                acc1 = psump.tile([P, F], dt.float32, tag="acc1")
                acc2 = psump.tile([P, F], dt.float32, tag="acc2")
                for cs in groups:
                    ng = len(cs)
                    c0 = cs[0]
                    lt = lp.tile([P, CG * F], dt.float32, tag="lt")
                    nc.sync.dma_start(
                        lt[:, : ng * Fp].rearrange("p (c f) -> p c f", f=Fp),
                        lgv[:, c0 : c0 + ng, :],
                    )
                    st = sp.tile([P, CG * F], dt.float32, tag="st")
                    nc.sync.dma_start(
                        st[:, : ng * Fp].rearrange("p (c f) -> p c f", f=Fp),
                        smv[:, c0 : c0 + ng, :],
                    )

                    for j, c in enumerate(cs):
                        lsl = lt[:, j * Fp : (j + 1) * Fp]
                        ssl = st[:, j * Fp : (j + 1) * Fp]

                        et = ep.tile([P, F], dt.bfloat16, tag="et")
                        nc.scalar.activation(et[:, :Fp], lsl, AF.Exp)

                        swt = swp.tile([P, F], dt.bfloat16, tag="swt")
                        nc.vector.tensor_scalar(
                            swt[:, :Fp], ssl, wr_t[:, c : c + 1], None, OP.mult
                        )

                        mt = mp.tile([P, F], dt.bfloat16, tag="mt")
                        nc.vector.scalar_tensor_tensor(
                            mt[:, :Fp], ssl, wr_t[:, c : c + 1], lsl, OP.mult, OP.mult
                        )

                        first, last = c == 0, c == C - 1
                        nc.tensor.matmul(
                            acc_e[:, :Fp], id_t[:], et[:, :Fp], start=first, stop=last
                        )
                        nc.tensor.matmul(
                            acc1[:, :Fp], id_t[:], swt[:, :Fp], start=first, stop=last
                        )
                        nc.tensor.matmul(
                            acc2[:, :Fp], id_t[:], mt[:, :Fp], start=first, stop=last
                        )

                lse = outp.tile([P, F], dt.float32, tag="lse")
                nc.scalar.activation(lse[:, :Fp], acc_e[:, :Fp], AF.Ln)
                prod = outp.tile([P, F], dt.float32, tag="prod")
                nc.vector.tensor_tensor(prod[:, :Fp], lse[:, :Fp], acc1[:, :Fp], OP.mult)
                lo = outp.tile([P, F], dt.float32, tag="lo")
                nc.vector.tensor_tensor(lo[:, :Fp], prod[:, :Fp], acc2[:, :Fp], OP.subtract)
                # issue from gpsimd: an SP-issued output DMA would make the
                # in-order SP sequencer block on the loss-ready sem and stall
                # the next position's input DMA issues (head-of-line blocking)
                nc.gpsimd.dma_start(lov, lo[:, :Fp])
                pix_off += npx

    nc.finalize()
    return nc


def _get_nc():
    if "nc" not in _cache:
        _cache["nc"] = build_nc()
    return _cache["nc"]


def _shards(logits, smooth_labels):
    """Split on (b, h-half): core i <- b=i//2, hh=i%2, as [C, PIX_PER_CORE]."""
    lgs, sms = [], []
    for i in range(NCORES):
        b, hh = divmod(i, 2)
        h0 = hh * (H // 2)
        lgs.append(
            np.ascontiguousarray(logits[b, :, h0 : h0 + H // 2, :]).reshape(
                C, PIX_PER_CORE
            )
        )
        sms.append(
            np.ascontiguousarray(smooth_labels[b, :, h0 : h0 + H // 2, :]).reshape(
                C, PIX_PER_CORE
            )
        )
    return lgs, sms


def kernel(logits, labels, smooth_labels, weight2):
    import ml_dtypes
    from concourse.bass_utils import run_bass_kernel_spmd

    logits = np.asarray(logits, dtype=np.float32)
    smooth_labels = np.asarray(smooth_labels, dtype=np.float32)
    weight2 = np.asarray(weight2, dtype=np.float32)

    nc = _get_nc()
    lgs, sms = _shards(logits, smooth_labels)
    wrep = np.ascontiguousarray(np.broadcast_to(weight2, (P, C)))
    ident = np.eye(P, dtype=ml_dtypes.bfloat16)

    in_maps = [
        {"lg": lgs[i], "sm": sms[i], "wrep": wrep, "ident": ident}
        for i in range(NCORES)
    ]
    res = run_bass_kernel_spmd(nc, in_maps, list(range(NCORES)))
    flat = np.concatenate(
        [np.asarray(res.results[i]["loss"]).astype(np.float32) for i in range(NCORES)]
    )

    part = np.partition(flat, NPIX - K_TOP)
    topk = part[NPIX - K_TOP :]
    return np.asarray(topk.mean(dtype=np.float64), dtype=np.float32)
